# revision 29
# baseline (speedup 1.0000x reference)
"""CapsuleLayer dynamic-routing kernel for Trainium2 (8 NeuronCores).

Problem: inputs [B=32, I=2048, J=16], W [N=64, I=2048, D=32, J=16], routings=3.
  inputs_hat[b,n,i,d] = sum_j inputs[b,i,j] * W[n,i,d,j]
  3 rounds of routing (softmax over n, weighted sum over i, squash over d).

Strategy: shard the input-capsule axis I across the 8 cores (I_loc=256).
Each core recomputes its ihat shard from W each round (W streamed from HBM
as a single bf16 tensor; ihat never hits DRAM), keeps its b-state
[*, n, i_loc] in SBUF, and the only cross-core data is the [B, N, D]
partial sum s, AllReduced (256 KB) once per round.

Numerics: one bf16 product (xh*Wh) everywhere; bf16 H storage and bf16
elementwise products; f32 PSUM accumulation, f32 softmax/squash. Measured
(numpy sim of exact scheme): rel err ~4.4e-3 (gate 2e-2); extra products
do not help because the bf16-W logits path dominates the error.

Per-round-per-group (4 capsules) pipeline:
  PE: one K=64 block-diag matmul (lhsT = block-diag x, rhs = 64 contiguous
      W rows) per 1024-wide half -> PSUM H [128=(4c,32b), 2048=(64n,32d)]
  ACT: copy H halves PSUM->SBUF (bf16)
  DVE: ty = H*v (bf16 2x); y = reduce_d ty; b += y; softmax; c (bf16)
  DVE/GPS: tmp2 = c*H (bf16)
  PE: s_psum += sel.T @ tmp2  (folds 4 capsules AND b-diagonal, F=1024)
"""

import sys

for p in ("/opt/trn_rl_repo",):
    if p not in sys.path:
        sys.path.insert(0, p)

import ml_dtypes
import numpy as np

import concourse.bacc as bacc
import concourse.mybir as mybir
import concourse.tile as tile
from concourse.bass_utils import run_bass_kernel_spmd

# problem constants (hardcoded per harness contract)
B, N, I, D, J = 32, 64, 2048, 32, 16
R = 3  # routings
CORES = 8
I_LOC = I // CORES  # 256
ND = N * D  # 2048
EPS = 1e-7

F32 = mybir.dt.float32
BF16 = mybir.dt.bfloat16
FX = mybir.AxisListType.X
ADD = mybir.AluOpType.add
ACT = mybir.ActivationFunctionType

GROUPS = I_LOC // 4  # 64 groups of 4 capsules per round
CHUNKS = GROUPS // 2  # 32 W chunks of 128 rows (2 groups each)
HF = ND // 2  # 1024
DEBUG_DUMP = False  # dump round-1 intermediates for groups 0/1


def _squash_build(nc, vbpool, smalls, kp, s4, eps_ap, out_dtype):
    """s4: [128, 2048] tile holding s in (d n)-major order (replicated x4 on
    partition groups). Returns vb [128, 2048] = squash(s), same layout."""
    s2 = kp.tile([128, ND], F32, tag="tmp")
    nc.scalar.square(s2[:], s4[:])
    sq = smalls.tile([128, N], F32, tag="sq_sq")
    nc.vector.tensor_reduce(
        sq[:], s2[:].rearrange("p (d n) -> p n d", d=D), axis=FX, op=ADD)
    # t = sqrt(sq + eps)
    t = smalls.tile([128, N], F32, tag="sq_t")
    nc.scalar.activation(t[:], sq[:], ACT.Sqrt, bias=eps_ap)
    # q1 = 1 + sq
    q1 = smalls.tile([128, N], F32, tag="sq_q1")
    nc.scalar.activation(q1[:], sq[:], ACT.Identity, bias=1.0)
    den = smalls.tile([128, N], F32, tag="sq_den")
    nc.vector.tensor_mul(den[:], q1[:], t[:])
    rs = smalls.tile([128, N], F32, tag="sq_rs")
    nc.vector.reciprocal(rs[:], den[:])
    scale = smalls.tile([128, N], F32, tag="sq_scale")
    nc.vector.tensor_mul(scale[:], sq[:], rs[:])
    vb = vbpool.tile([128, ND], out_dtype, tag="sq_vb")
    nc.vector.tensor_mul(
        vb[:].rearrange("p (d n) -> p d n", d=D),
        s4[:].rearrange("p (d n) -> p d n", d=D),
        scale[:, None, :].broadcast_to([128, D, N]),
    )
    return vb


def build_kernel():
    nc = bacc.Bacc("TRN2", target_bir_lowering=False, debug=False)

    xth = nc.dram_tensor("xth", [I_LOC * J, B], BF16, kind="ExternalInput")
    xblk = nc.dram_tensor("xblk", [128, CHUNKS * 128], BF16,
                          kind="ExternalInput")
    wth = nc.dram_tensor("wth", [I_LOC * J, ND], BF16, kind="ExternalInput")
    out = nc.dram_tensor("out", [B, N, D], F32, kind="ExternalOutput")

    # collective bounce buffers (one pair per round)
    s_in = [nc.dram_tensor(f"s_in{r}", [B, ND], F32) for r in range(R)]
    s_out = [nc.dram_tensor(f"s_out{r}", [B, ND], F32, addr_space="Shared")
             for r in range(R)]
    if DEBUG_DUMP:
        dbg_h = nc.dram_tensor("dbg_h", [4, 128, ND], F32,
                               kind="ExternalOutput")
        dbg_y = nc.dram_tensor("dbg_y", [4, 128, N], F32,
                               kind="ExternalOutput")
        dbg_v = nc.dram_tensor("dbg_v", [128, ND], F32,
                               kind="ExternalOutput")
        dbg_c = nc.dram_tensor("dbg_c", [4, 128, N], F32,
                               kind="ExternalOutput")
        dbg_t2 = nc.dram_tensor("dbg_t2", [2, 128, ND], F32,
                                kind="ExternalOutput")
        dbg_s = nc.dram_tensor("dbg_s", [2, B, ND], F32,
                               kind="ExternalOutput")

    with tile.TileContext(nc) as tc:
        with (
            tc.tile_pool(name="persist", bufs=1) as pp,
            tc.tile_pool(name="wsbp", bufs=4) as wsbp,
            tc.tile_pool(name="vbp", bufs=2) as vbp,
            tc.tile_pool(name="work", bufs=2) as kp,
            tc.tile_pool(name="hsbp", bufs=8) as hsbp,
            tc.tile_pool(name="typ", bufs=4) as typ,
            tc.tile_pool(name="t2p", bufs=4) as t2p,
            tc.tile_pool(name="s4p", bufs=1) as s4p,
            tc.tile_pool(name="pbig", bufs=1) as pbig,
            tc.tile_pool(name="small", bufs=3) as sp,
            tc.tile_pool(name="psum", bufs=4, space="PSUM") as psp,
            tc.tile_pool(name="psumB", bufs=1, space="PSUM") as psB,
        ):
            # ---- resident tiles ----
            # x chunks for round-0 fused einsum: [128=(8i,16j), 32 chunks, B]
            xsbh = pp.tile([128, I_LOC * J // 128, B], BF16, tag="xsbh")
            nc.sync.dma_start(
                xsbh[:], xth[:].rearrange("(k p) b -> p k b", p=128))
            # block-diag x for per-group matmuls: [128, CHUNKS*128]
            xbs = pp.tile([128, CHUNKS * 128], BF16, tag="xbs")
            nc.sync.dma_start(xbs[:], xblk[:])

            # routing logits b: [128=(c,b), GROUPS, N]
            bstate = pp.tile([128, GROUPS, N], F32, tag="bstate")
            nc.gpsimd.memset(bstate[:], 0.0)
            eps_t = pp.tile([128, 1], F32, tag="eps")
            nc.gpsimd.memset(eps_t[:], EPS)
            # selector[p, m] = 1.0 if p % 32 == m  (partition-group fold)
            sel_i = pp.tile([128, B], mybir.dt.int32, tag="sel_i")
            nc.gpsimd.iota(sel_i[:], [[1, B]], channel_multiplier=-1)
            nc.vector.tensor_scalar(sel_i[:], sel_i[:], 31, None,
                                    op0=mybir.AluOpType.bitwise_and)
            sel = pp.tile([128, B], BF16, tag="sel")
            nc.vector.tensor_scalar(sel[:], sel_i[:], 0, None,
                                    op0=mybir.AluOpType.is_equal)

            # ---------- round 0: c uniform -> s0 = (1/N) sum_i ihat ----------
            ps0 = psB.tile([B, ND], F32, tag="pss")
            for k in range(CHUNKS):
                wsb = wsbp.tile([128, ND], BF16, tag="wsb")
                nc.sync.dma_start(wsb[:], wth[k * 128:(k + 1) * 128, :])
                for q in range(4):
                    nc.tensor.matmul(
                        ps0[:, q * 512:(q + 1) * 512],
                        xsbh[:, k, :],
                        wsb[:, q * 512:(q + 1) * 512],
                        start=(k == 0),
                        stop=(k == CHUNKS - 1),
                    )
            s_loc0 = pbig.tile([B, ND], F32, tag="s_loc")
            nc.scalar.mul(s_loc0[:], ps0[:], 1.0 / N)
            nc.sync.dma_start(s_in[0][:], s_loc0[:])
            nc.gpsimd.collective_compute(
                "AllReduce", ADD,
                replica_groups=[list(range(CORES))],
                ins=[s_in[0].ap().opt()], outs=[s_out[0].ap().opt()],
            )
            s4 = s4p.tile([128, ND], F32, tag="s4")
            for g4 in range(4):
                nc.sync.dma_start(s4[g4 * 32:(g4 + 1) * 32, :], s_out[0][:])
            vb = _squash_build(nc, vbp, sp, kp, s4, eps_t[:], BF16)
            if DEBUG_DUMP:
                vf32 = kp.tile([128, ND], F32, tag="dbg_vf32")
                nc.vector.tensor_copy(vf32[:], vb[:])
                nc.sync.dma_start(dbg_v[:], vf32[:])

            # ---------- rounds 1, 2 ----------
            for r in (1, 2):
                ps_s = psB.tile([B, ND], F32, tag="pss")
                pending = []  # previous groups' tmp2 (fold delayed 2 groups)

                def flush_fold(pend, last, _ps=ps_s):
                    g0, t2 = pend
                    for q in range(4):
                        nc.tensor.matmul(
                            _ps[:, q * 512:(q + 1) * 512],
                            sel[:],
                            t2[:, q * 512:(q + 1) * 512],
                            start=(g0 == 0),
                            stop=(last and q == 3),
                            skip_group_check=True,
                        )

                post = []  # groups whose softmax/tmp2 stage is deferred

                def stage_b(g, hsb):
                    # softmax over n (|b| is O(10): exp fine in f32)
                    bsl = bstate[:, g, :]
                    e = sp.tile([128, N], F32, tag="e")
                    se = sp.tile([128, 1], F32, tag="se")
                    nc.scalar.activation(e[:], bsl, ACT.Exp,
                                         accum_out=se[:])
                    rcp = sp.tile([128, 1], F32, tag="rcp")
                    nc.vector.reciprocal(rcp[:], se[:])
                    cg = sp.tile([128, N], BF16, tag="cg")
                    nc.vector.tensor_scalar_mul(cg[:], e[:], rcp[:])
                    # tmp2 = c * H  (folded into ps_s two groups later).
                    # (d n)-major layout: c broadcasts over the OUTER d axis,
                    # the inner n axis stays stride-1 -> 2x DVE rate.
                    t2 = t2p.tile([128, ND], BF16, tag="tmp2")
                    nc.vector.tensor_mul(
                        t2[:].rearrange("p (d n) -> p d n", d=D),
                        hsb[:].rearrange("p (d n) -> p d n", d=D),
                        cg[:, None, :].broadcast_to([128, D, N]),
                    )
                    pending.append((g, t2))

                for g in range(GROUPS):
                    k, half = g // 2, g % 2
                    if half == 0:
                        wsb = wsbp.tile([128, ND], BF16, tag="wsb")
                        nc.sync.dma_start(wsb[:],
                                          wth[k * 128:(k + 1) * 128, :])
                    lhs = xbs[64 * half:64 * half + 64,
                              128 * k:128 * (k + 1)]
                    # per-quarter MM -> copy: releases PSUM banks quickly so
                    # the PE stays dense (HAM stays warm)
                    hsb = hsbp.tile([128, ND], BF16, tag="hsb")
                    for q4 in range(4):
                        pg = psp.tile([128, 512], F32, tag="pg")
                        nc.tensor.matmul(
                            pg[:],
                            lhs,
                            wsb[64 * half:64 * half + 64,
                                q4 * 512:(q4 + 1) * 512],
                            start=True, stop=True,
                        )
                        nc.scalar.copy(hsb[:, q4 * 512:(q4 + 1) * 512], pg[:])
                    if DEBUG_DUMP and r == 1 and g < 4:
                        hf32 = kp.tile([128, ND], F32, tag="dbg_hf32")
                        nc.vector.tensor_copy(hf32[:], hsb[:])
                        nc.sync.dma_start(dbg_h[g], hf32[:])
                    # fold tmp2 from two stage_b's back
                    if len(pending) >= 2:
                        flush_fold(pending.pop(0), False)
                    # y = sum_d H * v   (bf16 mul at 2x rate, one op)
                    # y = sum_d H*v  (mul split DVE/GPS; one 1x reduce)
                    ty = typ.tile([128, ND], BF16, tag="ty")
                    nc.vector.tensor_mul(ty[:, 0:HF], hsb[:, 0:HF],
                                         vb[:, 0:HF])
                    nc.gpsimd.tensor_mul(ty[:, HF:ND], hsb[:, HF:ND],
                                         vb[:, HF:ND])
                    y = sp.tile([128, N], BF16, tag="y")
                    with nc.allow_low_precision(reason="y feeds f32 b-add"):
                        nc.vector.tensor_reduce(
                            y[:], ty[:].rearrange("p (d n) -> p n d", d=D),
                            axis=FX, op=ADD)
                    # b += y
                    bsl = bstate[:, g, :]
                    nc.vector.tensor_add(bsl, bsl, y[:])
                    # deferred softmax/tmp2 for the previous group
                    post.append((g, hsb))
                    if len(post) >= 2:
                        stage_b(*post.pop(0))
                stage_b(*post.pop(0))
                while len(pending) > 1:
                    flush_fold(pending.pop(0), False)
                flush_fold(pending.pop(0), True)

                s_loc = pbig.tile([B, ND], F32, tag="s_loc")
                nc.scalar.copy(s_loc[:], ps_s[:])
                if DEBUG_DUMP:
                    nc.sync.dma_start(dbg_s[r - 1], s_loc[:])
                nc.sync.dma_start(s_in[r][:], s_loc[:])
                nc.gpsimd.collective_compute(
                    "AllReduce", ADD,
                    replica_groups=[list(range(CORES))],
                    ins=[s_in[r].ap().opt()], outs=[s_out[r].ap().opt()],
                )
                s4 = s4p.tile([128, ND], F32, tag="s4")
                for g4 in range(4):
                    nc.sync.dma_start(s4[g4 * 32:(g4 + 1) * 32, :],
                                      s_out[r][:])
                vb = _squash_build(nc, vbp, sp, kp, s4, eps_t[:],
                                   BF16 if r == 1 else F32)

            # output = squash(s2) = vb rows 0..31; vb is (d n)-major, so
            # transpose on-chip to (n d) before the contiguous DMA out
            vout = kp.tile([32, ND], F32, tag="vout")
            nc.vector.tensor_copy(
                vout[:].rearrange("p (n d) -> p n d", d=D),
                vb[0:32, :].rearrange("p (d n) -> p n d", d=D))
            nc.sync.dma_start(
                out[:].rearrange("b n d -> b (n d)"), vout[:])

    nc.compile()
    return nc


_NC_CACHE = {}


def _get_nc():
    if "nc" not in _NC_CACHE:
        _NC_CACHE["nc"] = build_kernel()
    return _NC_CACHE["nc"]


def _make_in_maps(inputs, W):
    inputs = np.ascontiguousarray(np.asarray(inputs, dtype=np.float32))
    W = np.ascontiguousarray(np.asarray(W, dtype=np.float32))
    assert inputs.shape == (B, I, J) and W.shape == (N, I, D, J)
    xh_all = inputs.astype(ml_dtypes.bfloat16)
    in_maps = []
    for c in range(CORES):
        sl = slice(c * I_LOC, (c + 1) * I_LOC)
        xh = xh_all[:, sl, :]  # [B, I_LOC, J] bf16
        # xt: [(i j), b]
        x_t = np.ascontiguousarray(
            xh.transpose(1, 2, 0).reshape(I_LOC * J, B))
        # block-diag lhsT: xblk[64*half + 16*c' + j, 128*k + 32*c + b]
        #   = xh[b, 4*(2k+half)+c, j] iff c' == c
        blk = np.zeros((128, CHUNKS * 128), dtype=ml_dtypes.bfloat16)
        for half in range(2):
            for cc in range(4):
                # [J, B, CHUNKS] slab for capsules i = 4*(2k+half)+cc
                caps = xh[:, 4 * half + cc::8, :]  # [B, CHUNKS, J]
                dst = blk[64 * half + 16 * cc:64 * half + 16 * cc + 16, :]
                dst = dst.reshape(16, CHUNKS, 128)
                dst[:, :, 32 * cc:32 * cc + 32] = caps.transpose(2, 1, 0)
        # wt: [(i j), (d n)] d-major ; wt[(i,j),(d,n)] = W[n, i, d, j]
        w_t = np.ascontiguousarray(
            W[:, sl, :, :].transpose(1, 3, 2, 0).reshape(I_LOC * J, ND)
        ).astype(ml_dtypes.bfloat16)
        in_maps.append({"xth": np.ascontiguousarray(x_t),
                        "xblk": np.ascontiguousarray(blk),
                        "wth": np.ascontiguousarray(w_t)})
    return in_maps


def _ensure_ntff_hook():
    """Register the axon NTFF profile hook if the image's antenv lacks it."""
    import types

    try:
        import antenv.axon_hooks  # noqa: F401
        return
    except ImportError:
        pass
    import antenv

    if "/root/.axon_site" not in sys.path:
        sys.path.insert(0, "/root/.axon_site")
    from trn_agent_boot.trn_boot import _ntff_profile_via_ctypes

    hook = {"h": _ntff_profile_via_ctypes("/opt/axon/libaxon_pjrt.so")}
    mod = types.ModuleType("antenv.axon_hooks")
    mod.get_axon_ntff_profile_hook = lambda: hook["h"]
    mod.set_axon_ntff_profile_hook = lambda h: hook.__setitem__("h", h)
    sys.modules["antenv.axon_hooks"] = mod
    antenv.axon_hooks = mod


def run(inputs, W, trace=False):
    nc = _get_nc()
    if trace:
        _ensure_ntff_hook()
        # zero-egress container: skip the artifact upload, keep files local
        import concourse.bass_utils as bu
        bu.upload_artifacts = lambda d: d
    res = run_bass_kernel_spmd(
        nc, _make_in_maps(inputs, W), core_ids=list(range(CORES)),
        trace=trace,
    )
    return res.results[0]["out"].reshape(B, N, D), res


def kernel(inputs, W, routings=R, **_unused):
    assert int(routings) == R
    out, _ = run(inputs, W, trace=False)
    return out


# revision 31
# speedup vs baseline: 1.0031x; 1.0031x over previous
"""CapsuleLayer dynamic-routing kernel for Trainium2 (8 NeuronCores).

Problem: inputs [B=32, I=2048, J=16], W [N=64, I=2048, D=32, J=16], routings=3.
  inputs_hat[b,n,i,d] = sum_j inputs[b,i,j] * W[n,i,d,j]
  3 rounds of routing (softmax over n, weighted sum over i, squash over d).

Strategy: shard the input-capsule axis I across the 8 cores (I_loc=256).
Each core recomputes its ihat shard from W each round (W streamed from HBM
as a single bf16 tensor; ihat never hits DRAM), keeps its b-state
[*, n, i_loc] in SBUF, and the only cross-core data is the [B, N, D]
partial sum s, AllReduced (256 KB) once per round.

Numerics: one bf16 product (xh*Wh) everywhere; bf16 H storage and bf16
elementwise products; f32 PSUM accumulation, f32 softmax/squash. Measured
(numpy sim of exact scheme): rel err ~4.4e-3 (gate 2e-2); extra products
do not help because the bf16-W logits path dominates the error.

Per-round-per-group (4 capsules) pipeline:
  PE: one K=64 block-diag matmul (lhsT = block-diag x, rhs = 64 contiguous
      W rows) per 1024-wide half -> PSUM H [128=(4c,32b), 2048=(64n,32d)]
  ACT: copy H halves PSUM->SBUF (bf16)
  DVE: ty = H*v (bf16 2x); y = reduce_d ty; b += y; softmax; c (bf16)
  DVE/GPS: tmp2 = c*H (bf16)
  PE: s_psum += sel.T @ tmp2  (folds 4 capsules AND b-diagonal, F=1024)
"""

import sys

for p in ("/opt/trn_rl_repo",):
    if p not in sys.path:
        sys.path.insert(0, p)

import ml_dtypes
import numpy as np

import concourse.bacc as bacc
import concourse.mybir as mybir
import concourse.tile as tile
from concourse.bass_utils import run_bass_kernel_spmd

# problem constants (hardcoded per harness contract)
B, N, I, D, J = 32, 64, 2048, 32, 16
R = 3  # routings
CORES = 8
I_LOC = I // CORES  # 256
ND = N * D  # 2048
EPS = 1e-7

F32 = mybir.dt.float32
BF16 = mybir.dt.bfloat16
FX = mybir.AxisListType.X
ADD = mybir.AluOpType.add
ACT = mybir.ActivationFunctionType

GROUPS = I_LOC // 4  # 64 groups of 4 capsules per round
CHUNKS = GROUPS // 2  # 32 W chunks of 128 rows (2 groups each)
HF = ND // 2  # 1024
DEBUG_DUMP = False  # dump round-1 intermediates for groups 0/1


def _squash_build(nc, vbpool, smalls, kp, s4, eps_ap, out_dtype):
    """s4: [128, 2048] tile holding s in (d n)-major order (replicated x4 on
    partition groups). Returns vb [128, 2048] = squash(s), same layout."""
    s2 = kp.tile([128, ND], F32, tag="tmp")
    nc.scalar.square(s2[:], s4[:])
    sq = smalls.tile([128, N], F32, tag="sq_sq")
    nc.vector.tensor_reduce(
        sq[:], s2[:].rearrange("p (d n) -> p n d", d=D), axis=FX, op=ADD)
    # t = sqrt(sq + eps)
    t = smalls.tile([128, N], F32, tag="sq_t")
    nc.scalar.activation(t[:], sq[:], ACT.Sqrt, bias=eps_ap)
    # q1 = 1 + sq
    q1 = smalls.tile([128, N], F32, tag="sq_q1")
    nc.scalar.activation(q1[:], sq[:], ACT.Identity, bias=1.0)
    den = smalls.tile([128, N], F32, tag="sq_den")
    nc.vector.tensor_mul(den[:], q1[:], t[:])
    rs = smalls.tile([128, N], F32, tag="sq_rs")
    nc.vector.reciprocal(rs[:], den[:])
    scale = smalls.tile([128, N], F32, tag="sq_scale")
    nc.vector.tensor_mul(scale[:], sq[:], rs[:])
    vb = vbpool.tile([128, ND], out_dtype, tag="sq_vb")
    nc.vector.tensor_mul(
        vb[:].rearrange("p (d n) -> p d n", d=D),
        s4[:].rearrange("p (d n) -> p d n", d=D),
        scale[:, None, :].broadcast_to([128, D, N]),
    )
    return vb


def build_kernel():
    nc = bacc.Bacc("TRN2", target_bir_lowering=False, debug=False)

    xth = nc.dram_tensor("xth", [I_LOC * J, B], BF16, kind="ExternalInput")
    xblk = nc.dram_tensor("xblk", [128, CHUNKS * 128], BF16,
                          kind="ExternalInput")
    wth = nc.dram_tensor("wth", [I_LOC * J, ND], BF16, kind="ExternalInput")
    out = nc.dram_tensor("out", [B, N, D], F32, kind="ExternalOutput")

    # collective bounce buffers (one pair per round)
    s_in = [nc.dram_tensor(f"s_in{r}", [B, ND], F32) for r in range(R)]
    s_out = [nc.dram_tensor(f"s_out{r}", [B, ND], F32, addr_space="Shared")
             for r in range(R)]
    if DEBUG_DUMP:
        dbg_h = nc.dram_tensor("dbg_h", [4, 128, ND], F32,
                               kind="ExternalOutput")
        dbg_y = nc.dram_tensor("dbg_y", [4, 128, N], F32,
                               kind="ExternalOutput")
        dbg_v = nc.dram_tensor("dbg_v", [128, ND], F32,
                               kind="ExternalOutput")
        dbg_c = nc.dram_tensor("dbg_c", [4, 128, N], F32,
                               kind="ExternalOutput")
        dbg_t2 = nc.dram_tensor("dbg_t2", [2, 128, ND], F32,
                                kind="ExternalOutput")
        dbg_s = nc.dram_tensor("dbg_s", [2, B, ND], F32,
                               kind="ExternalOutput")

    with tile.TileContext(nc) as tc:
        with (
            tc.tile_pool(name="persist", bufs=1) as pp,
            tc.tile_pool(name="wsbp", bufs=4) as wsbp,
            tc.tile_pool(name="vbp", bufs=2) as vbp,
            tc.tile_pool(name="work", bufs=2) as kp,
            tc.tile_pool(name="hsbp", bufs=8) as hsbp,
            tc.tile_pool(name="typ", bufs=4) as typ,
            tc.tile_pool(name="t2p", bufs=4) as t2p,
            tc.tile_pool(name="s4p", bufs=1) as s4p,
            tc.tile_pool(name="pbig", bufs=1) as pbig,
            tc.tile_pool(name="small", bufs=3) as sp,
            tc.tile_pool(name="psum", bufs=4, space="PSUM") as psp,
            tc.tile_pool(name="psumB", bufs=1, space="PSUM") as psB,
        ):
            # ---- resident tiles ----
            # x chunks for round-0 fused einsum: [128=(8i,16j), 32 chunks, B]
            xsbh = pp.tile([128, I_LOC * J // 128, B], BF16, tag="xsbh")
            nc.sync.dma_start(
                xsbh[:], xth[:].rearrange("(k p) b -> p k b", p=128))
            # block-diag x for per-group matmuls: [128, CHUNKS*128]
            xbs = pp.tile([128, CHUNKS * 128], BF16, tag="xbs")
            nc.sync.dma_start(xbs[:], xblk[:])

            # routing logits b: [128=(c,b), GROUPS, N]
            bstate = pp.tile([128, GROUPS, N], F32, tag="bstate")
            nc.gpsimd.memset(bstate[:], 0.0)
            eps_t = pp.tile([128, 1], F32, tag="eps")
            nc.gpsimd.memset(eps_t[:], EPS)
            # selector[p, m] = 1.0 if p % 32 == m  (partition-group fold)
            sel_i = pp.tile([128, B], mybir.dt.int32, tag="sel_i")
            nc.gpsimd.iota(sel_i[:], [[1, B]], channel_multiplier=-1)
            nc.vector.tensor_scalar(sel_i[:], sel_i[:], 31, None,
                                    op0=mybir.AluOpType.bitwise_and)
            sel = pp.tile([128, B], BF16, tag="sel")
            nc.vector.tensor_scalar(sel[:], sel_i[:], 0, None,
                                    op0=mybir.AluOpType.is_equal)

            # ---------- round 0: c uniform -> s0 = (1/N) sum_i ihat ----------
            ps0 = psB.tile([B, ND], F32, tag="pss")
            for k in range(CHUNKS):
                wsb = wsbp.tile([128, ND], BF16, tag="wsb")
                nc.sync.dma_start(wsb[:], wth[k * 128:(k + 1) * 128, :])
                for q in range(4):
                    nc.tensor.matmul(
                        ps0[:, q * 512:(q + 1) * 512],
                        xsbh[:, k, :],
                        wsb[:, q * 512:(q + 1) * 512],
                        start=(k == 0),
                        stop=(k == CHUNKS - 1),
                    )
            s_loc0 = pbig.tile([B, ND], F32, tag="s_loc")
            nc.scalar.mul(s_loc0[:], ps0[:], 1.0 / N)
            nc.sync.dma_start(s_in[0][:], s_loc0[:])
            nc.gpsimd.collective_compute(
                "AllReduce", ADD,
                replica_groups=[list(range(CORES))],
                ins=[s_in[0].ap().opt()], outs=[s_out[0].ap().opt()],
            )
            s4 = s4p.tile([128, ND], F32, tag="s4")
            for g4 in range(4):
                nc.sync.dma_start(s4[g4 * 32:(g4 + 1) * 32, :], s_out[0][:])
            vb = _squash_build(nc, vbp, sp, kp, s4, eps_t[:], BF16)
            if DEBUG_DUMP:
                vf32 = kp.tile([128, ND], F32, tag="dbg_vf32")
                nc.vector.tensor_copy(vf32[:], vb[:])
                nc.sync.dma_start(dbg_v[:], vf32[:])

            # ---------- rounds 1, 2 ----------
            for r in (1, 2):
                ps_s = psB.tile([B, ND], F32, tag="pss")
                pending = []  # previous groups' tmp2 (fold delayed 2 groups)

                def flush_fold(pend, last, _ps=ps_s):
                    g0, t2 = pend
                    for q in range(4):
                        nc.tensor.matmul(
                            _ps[:, q * 512:(q + 1) * 512],
                            sel[:],
                            t2[:, q * 512:(q + 1) * 512],
                            start=(g0 == 0),
                            stop=(last and q == 3),
                            skip_group_check=True,
                        )

                post = []  # groups whose softmax/tmp2 stage is deferred

                def stage_b(g, hsb):
                    # softmax over n (|b| is O(10): exp fine in f32)
                    bsl = bstate[:, g, :]
                    e = sp.tile([128, N], F32, tag="e")
                    se = sp.tile([128, 1], F32, tag="se")
                    nc.scalar.activation(e[:], bsl, ACT.Exp,
                                         accum_out=se[:])
                    rcp = sp.tile([128, 1], F32, tag="rcp")
                    nc.vector.reciprocal(rcp[:], se[:])
                    cg = sp.tile([128, N], BF16, tag="cg")
                    nc.gpsimd.tensor_scalar_mul(cg[:], e[:], rcp[:])
                    # tmp2 = c * H  (folded into ps_s two groups later; both
                    # consumers are deferred so GPS latency on h1 is hidden).
                    # (d n)-major layout: c broadcasts over the OUTER d axis,
                    # the inner n axis stays stride-1 -> 2x DVE rate on h0.
                    t2 = t2p.tile([128, ND], BF16, tag="tmp2")
                    HD = D // 2
                    nc.vector.tensor_mul(
                        t2[:, 0:HF].rearrange("p (d n) -> p d n", d=HD),
                        hsb[:, 0:HF].rearrange("p (d n) -> p d n", d=HD),
                        cg[:, None, :].broadcast_to([128, HD, N]),
                    )
                    nc.gpsimd.tensor_mul(
                        t2[:, HF:ND].rearrange("p (d n) -> p d n", d=HD),
                        hsb[:, HF:ND].rearrange("p (d n) -> p d n", d=HD),
                        cg[:, None, :].broadcast_to([128, HD, N]),
                    )
                    pending.append((g, t2))

                for g in range(GROUPS):
                    k, half = g // 2, g % 2
                    if half == 0:
                        wsb = wsbp.tile([128, ND], BF16, tag="wsb")
                        nc.sync.dma_start(wsb[:],
                                          wth[k * 128:(k + 1) * 128, :])
                    lhs = xbs[64 * half:64 * half + 64,
                              128 * k:128 * (k + 1)]
                    # per-quarter MM -> copy: releases PSUM banks quickly so
                    # the PE stays dense (HAM stays warm)
                    hsb = hsbp.tile([128, ND], BF16, tag="hsb")
                    for q4 in range(4):
                        pg = psp.tile([128, 512], F32, tag="pg")
                        nc.tensor.matmul(
                            pg[:],
                            lhs,
                            wsb[64 * half:64 * half + 64,
                                q4 * 512:(q4 + 1) * 512],
                            start=True, stop=True,
                        )
                        nc.scalar.copy(hsb[:, q4 * 512:(q4 + 1) * 512], pg[:])
                    if DEBUG_DUMP and r == 1 and g < 4:
                        hf32 = kp.tile([128, ND], F32, tag="dbg_hf32")
                        nc.vector.tensor_copy(hf32[:], hsb[:])
                        nc.sync.dma_start(dbg_h[g], hf32[:])
                    # fold tmp2 from two stage_b's back
                    if len(pending) >= 2:
                        flush_fold(pending.pop(0), False)
                    # y = sum_d H * v   (bf16 mul at 2x rate, one op)
                    # y = sum_d H*v  (flat 2x mul; one 1x reduce)
                    ty = typ.tile([128, ND], BF16, tag="ty")
                    nc.vector.tensor_mul(ty[:], hsb[:], vb[:])
                    y = sp.tile([128, N], BF16, tag="y")
                    with nc.allow_low_precision(reason="y feeds f32 b-add"):
                        nc.vector.tensor_reduce(
                            y[:], ty[:].rearrange("p (d n) -> p n d", d=D),
                            axis=FX, op=ADD)
                    # b += y
                    bsl = bstate[:, g, :]
                    nc.vector.tensor_add(bsl, bsl, y[:])
                    # deferred softmax/tmp2 for the previous group
                    post.append((g, hsb))
                    if len(post) >= 2:
                        stage_b(*post.pop(0))
                stage_b(*post.pop(0))
                while len(pending) > 1:
                    flush_fold(pending.pop(0), False)
                flush_fold(pending.pop(0), True)

                s_loc = pbig.tile([B, ND], F32, tag="s_loc")
                nc.scalar.copy(s_loc[:], ps_s[:])
                if DEBUG_DUMP:
                    nc.sync.dma_start(dbg_s[r - 1], s_loc[:])
                nc.sync.dma_start(s_in[r][:], s_loc[:])
                nc.gpsimd.collective_compute(
                    "AllReduce", ADD,
                    replica_groups=[list(range(CORES))],
                    ins=[s_in[r].ap().opt()], outs=[s_out[r].ap().opt()],
                )
                s4 = s4p.tile([128, ND], F32, tag="s4")
                for g4 in range(4):
                    nc.sync.dma_start(s4[g4 * 32:(g4 + 1) * 32, :],
                                      s_out[r][:])
                vb = _squash_build(nc, vbp, sp, kp, s4, eps_t[:],
                                   BF16 if r == 1 else F32)

            # output = squash(s2) = vb rows 0..31; vb is (d n)-major, so
            # transpose on-chip to (n d) before the contiguous DMA out
            vout = kp.tile([32, ND], F32, tag="vout")
            nc.vector.tensor_copy(
                vout[:].rearrange("p (n d) -> p n d", d=D),
                vb[0:32, :].rearrange("p (d n) -> p n d", d=D))
            nc.sync.dma_start(
                out[:].rearrange("b n d -> b (n d)"), vout[:])

    nc.compile()
    return nc


_NC_CACHE = {}


def _get_nc():
    if "nc" not in _NC_CACHE:
        _NC_CACHE["nc"] = build_kernel()
    return _NC_CACHE["nc"]


def _make_in_maps(inputs, W):
    inputs = np.ascontiguousarray(np.asarray(inputs, dtype=np.float32))
    W = np.ascontiguousarray(np.asarray(W, dtype=np.float32))
    assert inputs.shape == (B, I, J) and W.shape == (N, I, D, J)
    xh_all = inputs.astype(ml_dtypes.bfloat16)
    in_maps = []
    for c in range(CORES):
        sl = slice(c * I_LOC, (c + 1) * I_LOC)
        xh = xh_all[:, sl, :]  # [B, I_LOC, J] bf16
        # xt: [(i j), b]
        x_t = np.ascontiguousarray(
            xh.transpose(1, 2, 0).reshape(I_LOC * J, B))
        # block-diag lhsT: xblk[64*half + 16*c' + j, 128*k + 32*c + b]
        #   = xh[b, 4*(2k+half)+c, j] iff c' == c
        blk = np.zeros((128, CHUNKS * 128), dtype=ml_dtypes.bfloat16)
        for half in range(2):
            for cc in range(4):
                # [J, B, CHUNKS] slab for capsules i = 4*(2k+half)+cc
                caps = xh[:, 4 * half + cc::8, :]  # [B, CHUNKS, J]
                dst = blk[64 * half + 16 * cc:64 * half + 16 * cc + 16, :]
                dst = dst.reshape(16, CHUNKS, 128)
                dst[:, :, 32 * cc:32 * cc + 32] = caps.transpose(2, 1, 0)
        # wt: [(i j), (d n)] d-major ; wt[(i,j),(d,n)] = W[n, i, d, j]
        w_t = np.ascontiguousarray(
            W[:, sl, :, :].transpose(1, 3, 2, 0).reshape(I_LOC * J, ND)
        ).astype(ml_dtypes.bfloat16)
        in_maps.append({"xth": np.ascontiguousarray(x_t),
                        "xblk": np.ascontiguousarray(blk),
                        "wth": np.ascontiguousarray(w_t)})
    return in_maps


def _ensure_ntff_hook():
    """Register the axon NTFF profile hook if the image's antenv lacks it."""
    import types

    try:
        import antenv.axon_hooks  # noqa: F401
        return
    except ImportError:
        pass
    import antenv

    if "/root/.axon_site" not in sys.path:
        sys.path.insert(0, "/root/.axon_site")
    from trn_agent_boot.trn_boot import _ntff_profile_via_ctypes

    hook = {"h": _ntff_profile_via_ctypes("/opt/axon/libaxon_pjrt.so")}
    mod = types.ModuleType("antenv.axon_hooks")
    mod.get_axon_ntff_profile_hook = lambda: hook["h"]
    mod.set_axon_ntff_profile_hook = lambda h: hook.__setitem__("h", h)
    sys.modules["antenv.axon_hooks"] = mod
    antenv.axon_hooks = mod


def run(inputs, W, trace=False):
    nc = _get_nc()
    if trace:
        _ensure_ntff_hook()
        # zero-egress container: skip the artifact upload, keep files local
        import concourse.bass_utils as bu
        bu.upload_artifacts = lambda d: d
    res = run_bass_kernel_spmd(
        nc, _make_in_maps(inputs, W), core_ids=list(range(CORES)),
        trace=trace,
    )
    return res.results[0]["out"].reshape(B, N, D), res


def kernel(inputs, W, routings=R, **_unused):
    assert int(routings) == R
    out, _ = run(inputs, W, trace=False)
    return out


# revision 38
# speedup vs baseline: 1.2362x; 1.2324x over previous
"""CapsuleLayer dynamic-routing kernel for Trainium2 (8 NeuronCores).

Problem: inputs [B=32, I=2048, J=16], W [N=64, I=2048, D=32, J=16], routings=3.
  inputs_hat[b,n,i,d] = sum_j inputs[b,i,j] * W[n,i,d,j]
  3 rounds of routing (softmax over n, weighted sum over i, squash over d).

Strategy: shard the input-capsule axis I across the 8 cores (I_loc=256).
Each core recomputes its ihat shard from W each round (W streamed from HBM
as a single bf16 tensor; ihat never hits DRAM), keeps its b-state
[*, n, i_loc] in SBUF, and the only cross-core data is the [B, N, D]
partial sum s, AllReduced (256 KB) once per round.

Numerics: one bf16 product (xh*Wh) everywhere; bf16 H storage and bf16
elementwise products; f32 PSUM accumulation, f32 softmax/squash. Measured
(numpy sim of exact scheme): rel err ~4.4e-3 (gate 2e-2); extra products
do not help because the bf16-W logits path dominates the error.

Per-round-per-group (4 capsules) pipeline:
  PE: one K=64 block-diag matmul (lhsT = block-diag x, rhs = 64 contiguous
      W rows) per 1024-wide half -> PSUM H [128=(4c,32b), 2048=(64n,32d)]
  ACT: copy H halves PSUM->SBUF (bf16)
  DVE: ty = H*v (bf16 2x); y = reduce_d ty; b += y; softmax; c (bf16)
  DVE/GPS: tmp2 = c*H (bf16)
  PE: s_psum += sel.T @ tmp2  (folds 4 capsules AND b-diagonal, F=1024)
"""

import sys

for p in ("/opt/trn_rl_repo",):
    if p not in sys.path:
        sys.path.insert(0, p)

import ml_dtypes
import numpy as np

import concourse.bacc as bacc
import concourse.mybir as mybir
import concourse.tile as tile
from concourse.bass_utils import run_bass_kernel_spmd

# problem constants (hardcoded per harness contract)
B, N, I, D, J = 32, 64, 2048, 32, 16
R = 3  # routings
CORES = 8
I_LOC = I // CORES  # 256
ND = N * D  # 2048
EPS = 1e-7

F32 = mybir.dt.float32
BF16 = mybir.dt.bfloat16
FX = mybir.AxisListType.X
ADD = mybir.AluOpType.add
ACT = mybir.ActivationFunctionType

GROUPS = I_LOC // 4  # 64 groups of 4 capsules per round
CHUNKS = GROUPS // 2  # 32 W chunks of 128 rows (2 groups each)
HF = ND // 2  # 1024
DEBUG_DUMP = False  # dump round-1 intermediates for groups 0/1


def _squash_build(nc, vbpool, smalls, kp, s4, eps_ap, out_dtype):
    """s4: [128, 2048] tile holding s (replicated x4 on partition groups).
    Returns vb [128, 2048] = squash(s) broadcast tile (same replication)."""
    sq = smalls.tile([128, N], F32, tag="sq_sq")
    for h in range(2):
        s2 = kp.tile([128, HF], F32, tag="tmp")
        nc.scalar.square(s2[:], s4[:, h * HF:(h + 1) * HF])
        nc.vector.tensor_reduce(
            sq[:, 32 * h:32 * (h + 1)],
            s2[:].rearrange("p (n d) -> p n d", d=D), axis=FX, op=ADD)
    # t = sqrt(sq + eps)
    t = smalls.tile([128, N], F32, tag="sq_t")
    nc.scalar.activation(t[:], sq[:], ACT.Sqrt, bias=eps_ap)
    # q1 = 1 + sq
    q1 = smalls.tile([128, N], F32, tag="sq_q1")
    nc.scalar.activation(q1[:], sq[:], ACT.Identity, bias=1.0)
    den = smalls.tile([128, N], F32, tag="sq_den")
    nc.vector.tensor_mul(den[:], q1[:], t[:])
    rs = smalls.tile([128, N], F32, tag="sq_rs")
    nc.vector.reciprocal(rs[:], den[:])
    scale = smalls.tile([128, N], F32, tag="sq_scale")
    nc.vector.tensor_mul(scale[:], sq[:], rs[:])
    vb = vbpool.tile([128, ND], out_dtype, tag="sq_vb")
    nc.vector.tensor_mul(
        vb[:].rearrange("p (n d) -> p n d", d=D),
        s4[:].rearrange("p (n d) -> p n d", d=D),
        scale[:, :, None].broadcast_to([128, N, D]),
    )
    return vb


def build_kernel():
    nc = bacc.Bacc("TRN2", target_bir_lowering=False, debug=False)

    xth = nc.dram_tensor("xth", [I_LOC * J, B], BF16, kind="ExternalInput")
    xblk = nc.dram_tensor("xblk", [128, CHUNKS * 128], BF16,
                          kind="ExternalInput")
    wth = nc.dram_tensor("wth", [I_LOC * J, ND], BF16, kind="ExternalInput")
    out = nc.dram_tensor("out", [B, N, D], F32, kind="ExternalOutput")

    # collective bounce buffers (one pair per round)
    s_in = [nc.dram_tensor(f"s_in{r}", [B, ND], F32) for r in range(R)]
    s_out = [nc.dram_tensor(f"s_out{r}", [B, ND], F32, addr_space="Shared")
             for r in range(R)]
    if DEBUG_DUMP:
        dbg_h = nc.dram_tensor("dbg_h", [4, 128, ND], F32,
                               kind="ExternalOutput")
        dbg_y = nc.dram_tensor("dbg_y", [4, 128, N], F32,
                               kind="ExternalOutput")
        dbg_v = nc.dram_tensor("dbg_v", [128, ND], F32,
                               kind="ExternalOutput")
        dbg_c = nc.dram_tensor("dbg_c", [4, 128, N], F32,
                               kind="ExternalOutput")
        dbg_t2 = nc.dram_tensor("dbg_t2", [2, 128, ND], F32,
                                kind="ExternalOutput")
        dbg_s = nc.dram_tensor("dbg_s", [2, B, ND], F32,
                               kind="ExternalOutput")

    with tile.TileContext(nc) as tc:
        with (
            tc.tile_pool(name="persist", bufs=1) as pp,
            tc.tile_pool(name="wsbp", bufs=3) as wsbp,
            tc.tile_pool(name="vbp", bufs=2) as vbp,
            tc.tile_pool(name="work", bufs=2) as kp,
            tc.tile_pool(name="hsbp", bufs=4) as hsbp,
            tc.tile_pool(name="typ", bufs=2) as typ,
            tc.tile_pool(name="t2p", bufs=3) as t2p,
            tc.tile_pool(name="s4p", bufs=1) as s4p,
            tc.tile_pool(name="pbig", bufs=1) as pbig,
            tc.tile_pool(name="small", bufs=3) as sp,
            tc.tile_pool(name="psum", bufs=4, space="PSUM") as psp,
            tc.tile_pool(name="psumB", bufs=1, space="PSUM") as psB,
        ):
            # ---- resident tiles ----
            # x chunks for round-0 fused einsum: [128=(8i,16j), 32 chunks, B]
            xsbh = pp.tile([128, I_LOC * J // 128, B], BF16, tag="xsbh")
            nc.sync.dma_start(
                xsbh[:], xth[:].rearrange("(k p) b -> p k b", p=128))
            # block-diag x for per-group matmuls: [128, CHUNKS*128]
            xbs = pp.tile([128, CHUNKS * 128], BF16, tag="xbs")
            nc.sync.dma_start(xbs[:], xblk[:])

            # routing logits b: [128=(c,b), GROUPS, N]
            bstate = pp.tile([128, GROUPS, N], F32, tag="bstate")
            nc.gpsimd.memset(bstate[:], 0.0)
            eps_t = pp.tile([128, 1], F32, tag="eps")
            nc.gpsimd.memset(eps_t[:], EPS)
            # selector[p, m] = 1.0 if p % 32 == m  (partition-group fold)
            sel_i = pp.tile([128, B], mybir.dt.int32, tag="sel_i")
            nc.gpsimd.iota(sel_i[:], [[1, B]], channel_multiplier=-1)
            nc.vector.tensor_scalar(sel_i[:], sel_i[:], 31, None,
                                    op0=mybir.AluOpType.bitwise_and)
            sel = pp.tile([128, B], BF16, tag="sel")
            nc.vector.tensor_scalar(sel[:], sel_i[:], 0, None,
                                    op0=mybir.AluOpType.is_equal)

            # ---------- round 0: c uniform -> s0 = (1/N) sum_i ihat ----------
            ps0 = psB.tile([B, ND], F32, tag="pss")
            for k in range(CHUNKS):
                wsb = wsbp.tile([128, ND], BF16, tag="wsb")
                nc.sync.dma_start(wsb[:], wth[k * 128:(k + 1) * 128, :])
                for q in range(4):
                    nc.tensor.matmul(
                        ps0[:, q * 512:(q + 1) * 512],
                        xsbh[:, k, :],
                        wsb[:, q * 512:(q + 1) * 512],
                        start=(k == 0),
                        stop=(k == CHUNKS - 1),
                    )
            s_loc0 = pbig.tile([B, ND], F32, tag="s_loc")
            nc.scalar.mul(s_loc0[:], ps0[:], 1.0 / N)
            nc.sync.dma_start(s_in[0][:], s_loc0[:])
            nc.gpsimd.collective_compute(
                "AllReduce", ADD,
                replica_groups=[list(range(CORES))],
                ins=[s_in[0].ap().opt()], outs=[s_out[0].ap().opt()],
            )
            s4 = s4p.tile([128, ND], F32, tag="s4")
            for g4 in range(4):
                nc.sync.dma_start(s4[g4 * 32:(g4 + 1) * 32, :], s_out[0][:])
            vb = _squash_build(nc, vbp, sp, kp, s4, eps_t[:], BF16)
            if DEBUG_DUMP:
                vf32 = kp.tile([128, ND], F32, tag="dbg_vf32")
                nc.vector.tensor_copy(vf32[:], vb[:])
                nc.sync.dma_start(dbg_v[:], vf32[:])

            # ---------- rounds 1, 2 ----------
            for r in (1, 2):
                ps_s = psB.tile([B, ND], F32, tag="pss")
                pending = []  # previous groups' tmp2 (fold delayed 2 groups)

                def flush_fold(pend, last, _ps=ps_s):
                    g0, t2 = pend
                    for q in range(4):
                        nc.tensor.matmul(
                            _ps[:, q * 512:(q + 1) * 512],
                            sel[:],
                            t2[:, q * 512:(q + 1) * 512],
                            start=(g0 == 0),
                            stop=(last and q == 3),
                            skip_group_check=True,
                        )

                post = []  # groups whose softmax/tmp2 stage is deferred

                def stage_b(g, hsb):
                    # softmax over n (|b| is O(10): exp fine in f32)
                    bsl = bstate[:, g, :]
                    e = sp.tile([128, N], F32, tag="e")
                    se = sp.tile([128, 1], F32, tag="se")
                    nc.scalar.activation(e[:], bsl, ACT.Exp,
                                         accum_out=se[:])
                    rcp = sp.tile([128, 1], F32, tag="rcp")
                    nc.vector.reciprocal(rcp[:], se[:])
                    cg = sp.tile([128, N], BF16, tag="cg")
                    nc.vector.tensor_scalar_mul(cg[:], e[:], rcp[:])
                    # tmp2 = c * H  (folded into ps_s two groups later; both
                    # consumers are deferred so GPS latency on h1 is hidden)
                    t2 = t2p.tile([128, ND], BF16, tag="tmp2")
                    for h in range(2):
                        eng = nc.vector if h == 0 else nc.gpsimd
                        eng.tensor_mul(
                            t2[:, h * HF:(h + 1) * HF].rearrange(
                                "p (n d) -> p n d", d=D),
                            hsb[:, h * HF:(h + 1) * HF].rearrange(
                                "p (n d) -> p n d", d=D),
                            cg[:, 32 * h:32 * (h + 1), None].broadcast_to(
                                [128, 32, D]),
                        )
                    pending.append((g, t2))

                for g in range(GROUPS):
                    k, half = g // 2, g % 2
                    if half == 0:
                        wsb = wsbp.tile([128, ND], BF16, tag="wsb")
                        nc.sync.dma_start(wsb[:],
                                          wth[k * 128:(k + 1) * 128, :])
                    lhs = xbs[64 * half:64 * half + 64,
                              128 * k:128 * (k + 1)]
                    # per-quarter MM -> copy: releases PSUM banks quickly so
                    # the PE stays dense (HAM stays warm)
                    hsb = hsbp.tile([128, ND], BF16, tag="hsb")
                    for q4 in range(4):
                        pg = psp.tile([128, 512], F32, tag="pg")
                        nc.tensor.matmul(
                            pg[:],
                            lhs,
                            wsb[64 * half:64 * half + 64,
                                q4 * 512:(q4 + 1) * 512],
                            start=True, stop=True,
                        )
                        nc.scalar.copy(hsb[:, q4 * 512:(q4 + 1) * 512], pg[:])
                    if DEBUG_DUMP and r == 1 and g < 4:
                        hf32 = kp.tile([128, ND], F32, tag="dbg_hf32")
                        nc.vector.tensor_copy(hf32[:], hsb[:])
                        nc.sync.dma_start(dbg_h[g], hf32[:])
                    # fold tmp2 from two stage_b's back
                    if len(pending) >= 2:
                        flush_fold(pending.pop(0), False)
                    # y = sum_d H * v   (bf16 mul at 2x rate, one op)
                    # y = sum_d H*v  (flat 2x mul; one 1x reduce)
                    ty = typ.tile([128, ND], BF16, tag="ty")
                    nc.vector.tensor_mul(ty[:], hsb[:], vb[:])
                    y = sp.tile([128, N], BF16, tag="y")
                    with nc.allow_low_precision(reason="y feeds f32 b-add"):
                        nc.vector.tensor_reduce(
                            y[:], ty[:].rearrange("p (n d) -> p n d", d=D),
                            axis=FX, op=ADD)
                    # b += y
                    bsl = bstate[:, g, :]
                    nc.vector.tensor_add(bsl, bsl, y[:])
                    # deferred softmax/tmp2 for the previous group
                    post.append((g, hsb))
                    if len(post) >= 2:
                        stage_b(*post.pop(0))
                stage_b(*post.pop(0))
                while len(pending) > 1:
                    flush_fold(pending.pop(0), False)
                flush_fold(pending.pop(0), True)

                s_loc = pbig.tile([B, ND], F32, tag="s_loc")
                nc.scalar.copy(s_loc[:], ps_s[:])
                if DEBUG_DUMP:
                    nc.sync.dma_start(dbg_s[r - 1], s_loc[:])
                nc.sync.dma_start(s_in[r][:], s_loc[:])
                nc.gpsimd.collective_compute(
                    "AllReduce", ADD,
                    replica_groups=[list(range(CORES))],
                    ins=[s_in[r].ap().opt()], outs=[s_out[r].ap().opt()],
                )
                s4 = s4p.tile([128, ND], F32, tag="s4")
                for g4 in range(4):
                    nc.sync.dma_start(s4[g4 * 32:(g4 + 1) * 32, :],
                                      s_out[r][:])
                vb = _squash_build(nc, vbp, sp, kp, s4, eps_t[:],
                                   BF16 if r == 1 else F32)

            # output = squash(s2) = vb rows 0..31
            nc.sync.dma_start(
                out[:].rearrange("b n d -> b (n d)"), vb[0:32, :])

    nc.compile()
    return nc


_NC_CACHE = {}


def _get_nc():
    if "nc" not in _NC_CACHE:
        _NC_CACHE["nc"] = build_kernel()
    return _NC_CACHE["nc"]


def _make_in_maps(inputs, W):
    inputs = np.ascontiguousarray(np.asarray(inputs, dtype=np.float32))
    W = np.ascontiguousarray(np.asarray(W, dtype=np.float32))
    assert inputs.shape == (B, I, J) and W.shape == (N, I, D, J)
    xh_all = inputs.astype(ml_dtypes.bfloat16)
    in_maps = []
    for c in range(CORES):
        sl = slice(c * I_LOC, (c + 1) * I_LOC)
        xh = xh_all[:, sl, :]  # [B, I_LOC, J] bf16
        # xt: [(i j), b]
        x_t = np.ascontiguousarray(
            xh.transpose(1, 2, 0).reshape(I_LOC * J, B))
        # block-diag lhsT: xblk[64*half + 16*c' + j, 128*k + 32*c + b]
        #   = xh[b, 4*(2k+half)+c, j] iff c' == c
        blk = np.zeros((128, CHUNKS * 128), dtype=ml_dtypes.bfloat16)
        for half in range(2):
            for cc in range(4):
                # [J, B, CHUNKS] slab for capsules i = 4*(2k+half)+cc
                caps = xh[:, 4 * half + cc::8, :]  # [B, CHUNKS, J]
                dst = blk[64 * half + 16 * cc:64 * half + 16 * cc + 16, :]
                dst = dst.reshape(16, CHUNKS, 128)
                dst[:, :, 32 * cc:32 * cc + 32] = caps.transpose(2, 1, 0)
        # wt: [(i j), (n d)] ; wt[(i,j),(n,d)] = W[n, i, d, j]
        w_t = np.ascontiguousarray(
            W[:, sl, :, :].transpose(1, 3, 0, 2).reshape(I_LOC * J, ND)
        ).astype(ml_dtypes.bfloat16)
        in_maps.append({"xth": np.ascontiguousarray(x_t),
                        "xblk": np.ascontiguousarray(blk),
                        "wth": np.ascontiguousarray(w_t)})
    return in_maps


def _ensure_ntff_hook():
    """Register the axon NTFF profile hook if the image's antenv lacks it."""
    import types

    try:
        import antenv.axon_hooks  # noqa: F401
        return
    except ImportError:
        pass
    import antenv

    if "/root/.axon_site" not in sys.path:
        sys.path.insert(0, "/root/.axon_site")
    from trn_agent_boot.trn_boot import _ntff_profile_via_ctypes

    hook = {"h": _ntff_profile_via_ctypes("/opt/axon/libaxon_pjrt.so")}
    mod = types.ModuleType("antenv.axon_hooks")
    mod.get_axon_ntff_profile_hook = lambda: hook["h"]
    mod.set_axon_ntff_profile_hook = lambda h: hook.__setitem__("h", h)
    sys.modules["antenv.axon_hooks"] = mod
    antenv.axon_hooks = mod


def run(inputs, W, trace=False):
    nc = _get_nc()
    if trace:
        _ensure_ntff_hook()
        # zero-egress container: skip the artifact upload, keep files local
        import concourse.bass_utils as bu
        bu.upload_artifacts = lambda d: d
    res = run_bass_kernel_spmd(
        nc, _make_in_maps(inputs, W), core_ids=list(range(CORES)),
        trace=trace,
    )
    return res.results[0]["out"].reshape(B, N, D), res


def kernel(inputs, W, routings=R, **_unused):
    assert int(routings) == R
    out, _ = run(inputs, W, trace=False)
    return out
